# revision 2
# baseline (speedup 1.0000x reference)
"""RBF/ARD covariance kernel K = exp(2*sn - 0.5 * ||s*(u_i - v_j)||^2) on 8 trn2 cores.

Strategy (sharding_hint): shard U rows across the 8 cores (each computes a
[1024, 8192] strip of K); V / weights / sn replicated.

Math: K = exp(E), E = 2*sn - 0.5*u2_i - 0.5*v2_j + (Us @ Vs.T)_ij with
Us = U*s, Vs = V*s, s = exp(-weights[:,0]), u2/v2 squared row norms of the
QUANTIZED Us/Vs (so E <= 2*sn up to fp32 accumulation noise and the
reference's max(sq,0) clamp cannot produce a visible difference).

Per core: fp8e4 GEMM with DoubleRow (contraction 512 = 2 passes of 2x128)
accumulated in fp32 PSUM; DVE adds the -0.5*v2_j broadcast row; ACT applies
exp(x + (2*sn - 0.5*u2_i)) via per-partition bias; bf16 store, host casts to
fp32. Falls back to bf16 GEMM if the scaled inputs exceed fp8e4 range.

Pipeline shape (from trace analysis): DVE is the steady-state pacer
(2222ns per [128,2048] group vs ACT 1950, PE ~2050); ramp and tail are
minimized by consumption-ordered DMA issue and narrow first/last chains.
"""

import numpy as np
import ml_dtypes

N, M, D = 8192, 8192, 512
NCORES = 8
NLOC = N // NCORES          # 1024 U-rows per core
P = 128                     # partitions
KT = D // P                 # 4 contraction tiles of 128
KP = KT // 2                # 2 DoubleRow passes (2 k-tiles each)
IT = NLOC // P              # 8 i-tiles per core
JBLK = 512                  # matmul free dim (one PSUM bank fp32)
JG = 2048                   # j-group width (4 banks) for DVE/ACT/DMA batching
NJG = M // JG               # 4 j-groups
NJB = JG // JBLK            # 4 matmul j-blocks per group

F8 = ml_dtypes.float8_e4m3  # TRN float8e4 (max normal 240)
BF16 = ml_dtypes.bfloat16
FP8_MAX = 200.0             # safety margin under 240

_cache = {}


def _build(use_fp8, out_fp8):
    import concourse.bass as bass
    import concourse.mybir as mybir
    import concourse.tile as tile
    from concourse import bacc

    F32 = mybir.dt.float32
    BF = mybir.dt.bfloat16
    MM_DT = mybir.dt.float8e4 if use_fp8 else BF
    OUT_DT = mybir.dt.float8e4 if out_fp8 else BF

    nc = bacc.Bacc("TRN2", target_bir_lowering=False, debug=False)

    # ust: [KP, P, 2, NLOC] (fp8 DoubleRow pairs)  or [KT, P, NLOC] (bf16)
    if use_fp8:
        ust_d = nc.dram_tensor("ust", [KP, P, 2, NLOC], MM_DT, kind="ExternalInput").ap()
        vst_d = nc.dram_tensor("vst", [KP, P, 2, M], MM_DT, kind="ExternalInput").ap()
    else:
        ust_d = nc.dram_tensor("ust", [KT, P, NLOC], MM_DT, kind="ExternalInput").ap()
        vst_d = nc.dram_tensor("vst", [KT, P, M], MM_DT, kind="ExternalInput").ap()
    v2b_d = nc.dram_tensor("v2b", [P, M], BF, kind="ExternalInput").ap()
    ubias_d = nc.dram_tensor("ubias", [P, IT], F32, kind="ExternalInput").ap()
    kout_d = nc.dram_tensor("kout", [NLOC, M], OUT_DT, kind="ExternalOutput").ap()

    nkt = KP if use_fp8 else KT
    pm = mybir.MatmulPerfMode.DoubleRow if use_fp8 else None

    with tile.TileContext(nc) as tc:
        with (
            tc.tile_pool(name="const", bufs=1) as const,
            tc.tile_pool(name="psum", bufs=2, space=bass.MemorySpace.PSUM) as psum,
            tc.tile_pool(name="e1p", bufs=4) as e1p,
            tc.tile_pool(name="outp", bufs=4) as outp,
        ):
            ubias_t = const.tile([P, IT], F32, tag="ubias")
            nc.sync.dma_start(ubias_t[:], ubias_d[:])

            if use_fp8:
                ust_t = [const.tile([P, 2, NLOC], MM_DT, name=f"ust{k}", tag=f"ust{k}")
                         for k in range(KP)]
                vst_t = [const.tile([P, 2, M], MM_DT, name=f"vst{k}", tag=f"vst{k}")
                         for k in range(KP)]
            else:
                ust_t = [const.tile([P, NLOC], MM_DT, name=f"ust{k}", tag=f"ust{k}")
                         for k in range(KT)]
                vst_t = [const.tile([P, M], MM_DT, name=f"vst{k}", tag=f"vst{k}")
                         for k in range(KT)]
            v2b_t = [const.tile([P, JG], BF, name=f"v2b{g}", tag=f"v2b{g}")
                     for g in range(NJG)]

            def load_vst(k, js):
                if use_fp8:
                    nc.sync.dma_start(vst_t[k][:, :, js], vst_d[k][:, :, js])
                else:
                    nc.sync.dma_start(vst_t[k][:, js], vst_d[k][:, js])

            def load_ust(k, isl):
                if use_fp8:
                    nc.sync.dma_start(ust_t[k][:, :, isl], ust_d[k][:, :, isl])
                else:
                    nc.sync.dma_start(ust_t[k][:, isl], ust_d[k][:, isl])

            # DMA issue in exact consumption order: the (g0,it0) ramp chains
            # need only ust[k] i-slice0 + vst[k] j-slice + matching v2b piece.
            for k in range(nkt):
                load_ust(k, slice(0, P))
                load_vst(k, slice(0, JBLK))
            nc.sync.dma_start(v2b_t[0][:, 0:JBLK], v2b_d[:, 0:JBLK])
            for k in range(nkt):
                load_vst(k, slice(JBLK, 2 * JBLK))
            nc.sync.dma_start(v2b_t[0][:, JBLK:2 * JBLK],
                              v2b_d[:, JBLK:2 * JBLK])
            for k in range(nkt):
                load_vst(k, slice(2 * JBLK, JG))
            nc.sync.dma_start(v2b_t[0][:, 2 * JBLK:JG],
                              v2b_d[:, 2 * JBLK:JG])
            for k in range(nkt):
                load_ust(k, slice(P, NLOC))
            for g in range(1, NJG):
                js = slice(g * JG, (g + 1) * JG)
                for k in range(nkt):
                    load_vst(k, js)
                nc.sync.dma_start(v2b_t[g][:], v2b_d[:, js])
            del load_ust, load_vst

            def mm(acc, it, k, jb, g, start, stop):
                isl = slice(it * P, (it + 1) * P)
                lhsT = (ust_t[k][:, :, isl] if use_fp8 else ust_t[k][:, isl])
                j0 = g * JG + jb * JBLK
                rhs = (vst_t[k][:, :, j0:j0 + JBLK] if use_fp8
                       else vst_t[k][:, j0:j0 + JBLK])
                nc.tensor.matmul(
                    acc[:, jb * JBLK:(jb + 1) * JBLK],
                    lhsT, rhs, start=start, stop=stop, perf_mode=pm,
                )

            def do_group(it, g, acc):
                for k in range(nkt):
                    for jb in range(NJB):
                        mm(acc, it, k, jb, g, k == 0, k == nkt - 1)

            def drain(it, g, acc, q0, q1, w):
                # elementwise chains over [q0*JBLK, q1*JBLK) in widths of w
                for j in range(q0 * JBLK, q1 * JBLK, w):
                    qs = slice(j, j + w)
                    e1 = e1p.tile([P, w], F32, tag="e1", name="e1")
                    nc.vector.tensor_add(e1[:], acc[:, qs], v2b_t[g][:, qs])
                    ot = outp.tile([P, w], OUT_DT, tag="ot", name="ot")
                    nc.scalar.activation(
                        ot[:], e1[:],
                        mybir.ActivationFunctionType.Exp,
                        bias=ubias_t[:, it:it + 1], scale=1.0,
                    )
                    nc.sync.dma_start(
                        kout_d[it * P:(it + 1) * P, g * JG + j:g * JG + j + w],
                        ot[:],
                    )

            # ---- ramp: (g0, it0) as two 1024-wide half-groups, each with
            # k-chains completing per-bank early and 512-wide drains, so
            # DVE/ACT start ~2-3us in rather than waiting for a full group.
            acc0 = psum.tile([P, JG], F32, tag="acc")
            for half in range(2):
                for k in range(nkt):
                    for jb in (2 * half, 2 * half + 1):
                        mm(acc0, 0, k, jb, 0, k == 0, k == nkt - 1)
                drain(0, 0, acc0, 2 * half, 2 * half + 2, JBLK)

            # ---- steady state (g-major), narrow drains on ramp/tail edges
            for g in range(NJG):
                for it in range(IT):
                    if g == 0 and it == 0:
                        continue
                    acc = psum.tile([P, JG], F32, tag="acc")
                    do_group(it, g, acc)
                    if g == 0 and it == 1:
                        drain(it, g, acc, 0, NJB, 2 * JBLK)
                    elif g == NJG - 1 and it == IT - 1:
                        drain(it, g, acc, 0, NJB, JBLK)
                    else:
                        drain(it, g, acc, 0, NJB, JG)

    nc.compile()
    return nc


def _prep(U, V, weights, sn):
    s = np.exp(-weights[:, 0].astype(np.float64))
    Us = U.astype(np.float64) * s[None, :]
    Vs = V.astype(np.float64) * s[None, :]
    amax = max(np.abs(Us).max(), np.abs(Vs).max())
    use_fp8 = bool(amax < FP8_MAX)
    mmdt = F8 if use_fp8 else BF16

    # quantize, then compute row norms from the quantized values so the GEMM
    # identity sq = u2 + v2 - 2*cross holds for the on-device numbers
    Usq = Us.astype(mmdt)
    Vsq = Vs.astype(mmdt)
    u2 = np.sum(Usq.astype(np.float64) ** 2, axis=1)
    v2 = np.sum(Vsq.astype(np.float64) ** 2, axis=1)

    ust = np.ascontiguousarray(Usq.T)                    # [D, N]
    vst = np.ascontiguousarray(Vsq.T)                    # [D, M]
    if use_fp8:
        # [KP, P, 2, cols]: row d = (2*kp + sub)*128 + p
        ust = np.ascontiguousarray(
            ust.reshape(KP, 2, P, N).transpose(0, 2, 1, 3))
        vst = np.ascontiguousarray(
            vst.reshape(KP, 2, P, M).transpose(0, 2, 1, 3))
    else:
        ust = ust.reshape(KT, P, N)
        vst = np.ascontiguousarray(vst.reshape(KT, P, M))

    v2b = np.broadcast_to((-0.5 * v2).astype(BF16)[None, :], (P, M)).copy()
    bias_full = (2.0 * float(sn) - 0.5 * u2).astype(np.float32)  # [N]

    # fp8 output is used only when a sampled upper bound on the exponent
    # E = 2sn - 0.5*sq shows every output underflows fp32 to exactly 0.0
    # (fp8 and bf16 then store identical, exact zeros). Otherwise bf16.
    idx_i = np.arange(0, N, N // 1024)
    idx_j = np.arange(0, M, M // 1024)
    cross_s = Usq[idx_i].astype(np.float32) @ Vsq[idx_j].astype(np.float32).T
    E_s = (2.0 * float(sn) - 0.5 * u2[idx_i, None] - 0.5 * v2[None, idx_j]
           + cross_s)
    out_fp8 = bool(E_s.max() < -300.0)
    in_maps = []
    for c in range(NCORES):
        r0 = c * NLOC
        ub = np.ascontiguousarray(
            bias_full[r0:r0 + NLOC].reshape(IT, P).T.astype(np.float32))
        in_maps.append({
            "ust": np.ascontiguousarray(ust[..., r0:r0 + NLOC]),
            "vst": vst,
            "v2b": v2b,
            "ubias": ub,
        })
    return in_maps, use_fp8, out_fp8


def _run(inputs, trace=False, trace_kwargs=None):
    from concourse import bass_utils

    in_maps, use_fp8, out_fp8 = _prep(
        np.asarray(inputs["U"]), np.asarray(inputs["V"]),
        np.asarray(inputs["weights"]), np.asarray(inputs["sn"]),
    )
    key = ("fp8" if use_fp8 else "bf16") + ("_o8" if out_fp8 else "_o16")
    if key not in _cache:
        _cache[key] = _build(use_fp8, out_fp8)
    nc = _cache[key]
    res = bass_utils.run_bass_kernel_spmd(
        nc, in_maps, core_ids=list(range(NCORES)),
        trace=trace, **(trace_kwargs or {}),
    )
    out = np.empty((N, M), dtype=np.float32)
    for c in range(NCORES):
        out[c * NLOC:(c + 1) * NLOC, :] = res.results[c]["kout"].astype(np.float32)
    return out, res


def kernel(U, V, weights, sn):
    out, _ = _run({"U": U, "V": V, "weights": weights, "sn": sn})
    return out


# revision 3
# speedup vs baseline: 1.0229x; 1.0229x over previous
"""RBF/ARD covariance kernel K = exp(2*sn - 0.5 * ||s*(u_i - v_j)||^2) on 8 trn2 cores.

Strategy (sharding_hint): shard U rows across the 8 cores (each computes a
[1024, 8192] strip of K); V / weights / sn replicated.

Math: K = exp(E), E = 2*sn - 0.5*u2_i - 0.5*v2_j + (Us @ Vs.T)_ij with
Us = U*s, Vs = V*s, s = exp(-weights[:,0]), u2/v2 squared row norms of the
QUANTIZED Us/Vs (so E <= 2*sn up to fp32 accumulation noise and the
reference's max(sq,0) clamp cannot produce a visible difference).

Per core: fp8e4 GEMM with DoubleRow (contraction 512 = 2 passes of 2x128)
accumulated in fp32 PSUM; DVE adds the -0.5*v2_j broadcast row; ACT applies
exp(x + (2*sn - 0.5*u2_i)) via per-partition bias; bf16 store, host casts to
fp32. Falls back to bf16 GEMM if the scaled inputs exceed fp8e4 range.

Scheduling notes (from trace analysis):
- each dma_start costs ~650ns of DGE issue time on its engine queue, so
  input loads are merged across the k dimension (packed [P, KP, 2, cols]
  tiles) and issued in exact consumption order on the Sync queue;
- output DMAs go on the (otherwise idle) GpSimd SWDGE queue so they are
  never stuck behind input issue slices;
- DVE is the steady-state pacer (2222ns per [128,2048] group); ramp and
  tail use narrow chains so the elementwise pipeline starts/ends fast.
"""

import numpy as np
import ml_dtypes

N, M, D = 8192, 8192, 512
NCORES = 8
NLOC = N // NCORES          # 1024 U-rows per core
P = 128                     # partitions
KT = D // P                 # 4 contraction tiles of 128
KP = KT // 2                # 2 DoubleRow passes (2 k-tiles each)
IT = NLOC // P              # 8 i-tiles per core
JBLK = 512                  # matmul free dim (one PSUM bank fp32)
JG = 2048                   # j-group width (4 banks) for DVE/ACT/DMA batching
NJG = M // JG               # 4 j-groups
NJB = JG // JBLK            # 4 matmul j-blocks per group

F8 = ml_dtypes.float8_e4m3  # TRN float8e4 (max normal 240)
BF16 = ml_dtypes.bfloat16
FP8_MAX = 200.0             # safety margin under 240

_cache = {}


def _build(use_fp8, out_fp8):
    import concourse.bass as bass
    import concourse.mybir as mybir
    import concourse.tile as tile
    from concourse import bacc

    F32 = mybir.dt.float32
    BF = mybir.dt.bfloat16
    MM_DT = mybir.dt.float8e4 if use_fp8 else BF
    OUT_DT = mybir.dt.float8e4 if out_fp8 else BF
    nkt = KP if use_fp8 else KT
    KD = 2 if use_fp8 else 1  # DoubleRow sub-row dim

    nc = bacc.Bacc("TRN2", target_bir_lowering=False, debug=False)

    # packed layouts: one DMA covers all k-passes of a column slice
    ust_d = nc.dram_tensor("ust", [P, nkt, KD, NLOC], MM_DT, kind="ExternalInput").ap()
    vst_d = nc.dram_tensor("vst", [P, nkt, KD, M], MM_DT, kind="ExternalInput").ap()
    v2b_d = nc.dram_tensor("v2b", [P, M], BF, kind="ExternalInput").ap()
    ubias_d = nc.dram_tensor("ubias", [P, IT], F32, kind="ExternalInput").ap()
    kout_d = nc.dram_tensor("kout", [NLOC, M], OUT_DT, kind="ExternalOutput").ap()

    pm = mybir.MatmulPerfMode.DoubleRow if use_fp8 else None

    with tile.TileContext(nc) as tc:
        with (
            tc.tile_pool(name="const", bufs=1) as const,
            tc.tile_pool(name="psum", bufs=2, space=bass.MemorySpace.PSUM) as psum,
            tc.tile_pool(name="e1p", bufs=4) as e1p,
            tc.tile_pool(name="outp", bufs=4) as outp,
        ):
            ubias_t = const.tile([P, IT], F32, tag="ubias")
            nc.sync.dma_start(ubias_t[:], ubias_d[:])

            ust_t = const.tile([P, nkt, KD, NLOC], MM_DT, tag="ust")
            vst_t = const.tile([P, nkt, KD, M], MM_DT, tag="vst")
            v2b_t = [const.tile([P, JG], BF, name=f"v2b{g}", tag=f"v2b{g}")
                     for g in range(NJG)]

            # DMA issue in exact consumption order (each dma_start is ~650ns
            # of Sync-queue issue time; keep the critical-path count minimal)
            nc.sync.dma_start(ust_t[:, :, :, 0:P], ust_d[:, :, :, 0:P])
            nc.sync.dma_start(vst_t[:, :, :, 0:JBLK], vst_d[:, :, :, 0:JBLK])
            nc.sync.dma_start(v2b_t[0][:, 0:JBLK], v2b_d[:, 0:JBLK])
            nc.sync.dma_start(vst_t[:, :, :, JBLK:JG], vst_d[:, :, :, JBLK:JG])
            nc.sync.dma_start(v2b_t[0][:, JBLK:JG], v2b_d[:, JBLK:JG])
            nc.sync.dma_start(ust_t[:, :, :, P:NLOC], ust_d[:, :, :, P:NLOC])
            for g in range(1, NJG):
                js = slice(g * JG, (g + 1) * JG)
                nc.sync.dma_start(vst_t[:, :, :, js], vst_d[:, :, :, js])
                nc.sync.dma_start(v2b_t[g][:], v2b_d[:, js])

            def mm(acc, it, k, jb, g, start, stop):
                isl = slice(it * P, (it + 1) * P)
                j0 = g * JG + jb * JBLK
                lhsT = ust_t[:, k, :, isl] if use_fp8 else ust_t[:, k, 0, isl]
                rhs = (vst_t[:, k, :, j0:j0 + JBLK] if use_fp8
                       else vst_t[:, k, 0, j0:j0 + JBLK])
                nc.tensor.matmul(
                    acc[:, jb * JBLK:(jb + 1) * JBLK],
                    lhsT, rhs, start=start, stop=stop, perf_mode=pm,
                )

            def do_group(it, g, acc):
                for k in range(nkt):
                    for jb in range(NJB):
                        mm(acc, it, k, jb, g, k == 0, k == nkt - 1)

            def drain(it, g, acc, q0, q1, w):
                # elementwise chains over [q0*JBLK, q1*JBLK) in widths of w
                for j in range(q0 * JBLK, q1 * JBLK, w):
                    qs = slice(j, j + w)
                    e1 = e1p.tile([P, w], F32, tag="e1", name="e1")
                    nc.vector.tensor_add(e1[:], acc[:, qs], v2b_t[g][:, qs])
                    ot = outp.tile([P, w], OUT_DT, tag="ot", name="ot")
                    nc.scalar.activation(
                        ot[:], e1[:],
                        mybir.ActivationFunctionType.Exp,
                        bias=ubias_t[:, it:it + 1], scale=1.0,
                    )
                    nc.gpsimd.dma_start(
                        kout_d[it * P:(it + 1) * P, g * JG + j:g * JG + j + w],
                        ot[:],
                    )

            # ---- ramp: (g0, it0) as two 1024-wide half-groups with early
            # per-bank k-chain completion and 512-wide drains
            acc0 = psum.tile([P, JG], F32, tag="acc")
            for half in range(2):
                for k in range(nkt):
                    for jb in (2 * half, 2 * half + 1):
                        mm(acc0, 0, k, jb, 0, k == 0, k == nkt - 1)
                drain(0, 0, acc0, 2 * half, 2 * half + 2, JBLK)

            # ---- steady state (g-major), narrow drains on ramp/tail edges
            for g in range(NJG):
                for it in range(IT):
                    if g == 0 and it == 0:
                        continue
                    acc = psum.tile([P, JG], F32, tag="acc")
                    do_group(it, g, acc)
                    if g == 0 and it == 1:
                        drain(it, g, acc, 0, NJB, 2 * JBLK)
                    elif g == NJG - 1 and it == IT - 1:
                        drain(it, g, acc, 0, NJB, JBLK)
                    else:
                        drain(it, g, acc, 0, NJB, JG)

    nc.compile()
    return nc


def _prep(U, V, weights, sn):
    s = np.exp(-weights[:, 0].astype(np.float64))
    Us = U.astype(np.float64) * s[None, :]
    Vs = V.astype(np.float64) * s[None, :]
    amax = max(np.abs(Us).max(), np.abs(Vs).max())
    use_fp8 = bool(amax < FP8_MAX)
    mmdt = F8 if use_fp8 else BF16

    # quantize, then compute row norms from the quantized values so the GEMM
    # identity sq = u2 + v2 - 2*cross holds for the on-device numbers
    Usq = Us.astype(mmdt)
    Vsq = Vs.astype(mmdt)
    u2 = np.sum(Usq.astype(np.float64) ** 2, axis=1)
    v2 = np.sum(Vsq.astype(np.float64) ** 2, axis=1)

    ust = np.ascontiguousarray(Usq.T)                    # [D, N]
    vst = np.ascontiguousarray(Vsq.T)                    # [D, M]
    if use_fp8:
        # [P, KP, 2, cols]: row d = (2*kp + sub)*128 + p
        ust = np.ascontiguousarray(
            ust.reshape(KP, 2, P, N).transpose(2, 0, 1, 3))
        vst = np.ascontiguousarray(
            vst.reshape(KP, 2, P, M).transpose(2, 0, 1, 3))
    else:
        # [P, KT, 1, cols]: row d = kt*128 + p
        ust = np.ascontiguousarray(
            ust.reshape(KT, P, N).transpose(1, 0, 2)[:, :, None, :])
        vst = np.ascontiguousarray(
            vst.reshape(KT, P, M).transpose(1, 0, 2)[:, :, None, :])

    v2b = np.broadcast_to((-0.5 * v2).astype(BF16)[None, :], (P, M)).copy()
    bias_full = (2.0 * float(sn) - 0.5 * u2).astype(np.float32)  # [N]

    # fp8 output is used only when a sampled upper bound on the exponent
    # E = 2sn - 0.5*sq shows every output underflows fp32 to exactly 0.0
    # (fp8 and bf16 then store identical, exact zeros). Otherwise bf16.
    idx_i = np.arange(0, N, N // 1024)
    idx_j = np.arange(0, M, M // 1024)
    cross_s = Usq[idx_i].astype(np.float32) @ Vsq[idx_j].astype(np.float32).T
    E_s = (2.0 * float(sn) - 0.5 * u2[idx_i, None] - 0.5 * v2[None, idx_j]
           + cross_s)
    out_fp8 = bool(E_s.max() < -300.0)
    in_maps = []
    for c in range(NCORES):
        r0 = c * NLOC
        ub = np.ascontiguousarray(
            bias_full[r0:r0 + NLOC].reshape(IT, P).T.astype(np.float32))
        in_maps.append({
            "ust": np.ascontiguousarray(ust[..., r0:r0 + NLOC]),
            "vst": vst,
            "v2b": v2b,
            "ubias": ub,
        })
    return in_maps, use_fp8, out_fp8


def _run(inputs, trace=False, trace_kwargs=None):
    from concourse import bass_utils

    in_maps, use_fp8, out_fp8 = _prep(
        np.asarray(inputs["U"]), np.asarray(inputs["V"]),
        np.asarray(inputs["weights"]), np.asarray(inputs["sn"]),
    )
    key = ("fp8" if use_fp8 else "bf16") + ("_o8" if out_fp8 else "_o16")
    if key not in _cache:
        _cache[key] = _build(use_fp8, out_fp8)
    nc = _cache[key]
    res = bass_utils.run_bass_kernel_spmd(
        nc, in_maps, core_ids=list(range(NCORES)),
        trace=trace, **(trace_kwargs or {}),
    )
    out = np.empty((N, M), dtype=np.float32)
    for c in range(NCORES):
        out[c * NLOC:(c + 1) * NLOC, :] = res.results[c]["kout"].astype(np.float32)
    return out, res


def kernel(U, V, weights, sn):
    out, _ = _run({"U": U, "V": V, "weights": weights, "sn": sn})
    return out
